# revision 1
# baseline (speedup 1.0000x reference)
"""Trainium2 Bass kernel: 8 independent 3x3 filters applied to every channel.

Reference op: x[B=8, C=32, 224, 224], W[1, 8, 3, 3], Bv[8]
  -> y[B, 8*C, 222, 222],  y[b, d*C+c, i, j] = sum_{u,v} x[b,c,i+u,j+v] W[0,d,u,v] + Bv[d]

Sharding: data-parallel over batch B across the 8 cores (core k takes x[k]).

Per-core formulation (all compute on TensorE):
  Matmul M-columns are (d', rl) = 4 filters x 28 row-groups; each PSUM
  partition accumulates 4 consecutive output rows (r = base + 4*rl + t) via
  4 sub-row matmul groups t writing different PSUM column ranges:
    psum[(d'*28+rl), (img, t, j)] += sum_r LW[r, ...] * TILE[r, img, j+v]
  with LW[local(base+4rl+t)+u, ..., d'*28+rl] = W[0, 4dh+d', u, v] a banded
  weight matrix (band truncated at K=128; spill rows land in the 2 pad rows).
  K = 128 (full input tile on partitions, base 0), N = 444 = 2 images x 222
  (N>=256 keeps float32r matmuls at 1 cycle/row). The 3 v-shift matmuls
  accumulate in PSUM; bias is added during the PSUM->SBUF copy (DVE
  tensor_scalar). Each partition's accumulated (sb, t, j) block is fully
  contiguous in the permuted DRAM layout => 7104B DMA descriptors and one
  fully-contiguous 199KB DMA per output channel (host un-permutes rows).

Super-blocks: sb0 = output rows 0..111 from input tile rows 0:128,
sb1 = output rows 112..223 from input tile rows 96:224 (rows 222/223 are
garbage from band truncation and land in the 2 DRAM pad rows per channel).
"""

import os
import numpy as np

B, C, H, W_IN = 8, 32, 224, 224
ND, KS = 8, 3
HO, WO = 222, 222
NCORES = 8
GSZ = 8        # images per input-tile group
NRL = 28       # row-groups per super-block
NT = 4         # rows per row-group
MM = 4 * NRL   # matmul M (112)
TILE_ROWS = [(0, 128), (96, 128)]   # (dram row base, K)
SB_BASE = [0, 112]                  # output row base per super-block

_PROG_CACHE = {}


def _build(mode: str, n_imgs: int):
    """Build+compile the per-core Bass program.

    mode: 'f32' (exact, 4 cyc/row), 'f32r' (relaxed fp32, 1 cyc/row @ N>=256),
          'bf16' (host-cast inputs).
    """
    import concourse.mybir as mybir
    import concourse.tile as tile
    from concourse import bacc

    dt = mybir.dt
    if mode == "bf16":
        io_dt = dt.bfloat16
    elif mode == "f32r":
        io_dt = dt.float32r
    else:
        io_dt = dt.float32

    n_groups = n_imgs // GSZ
    assert n_imgs % GSZ == 0

    nc = bacc.Bacc("TRN2", target_bir_lowering=False, debug=False)
    xin = nc.dram_tensor("xin", [n_imgs, H, W_IN], io_dt, kind="ExternalInput")
    lw = nc.dram_tensor("lw", [128, 2, NT, 3, 2, MM], io_dt,
                        kind="ExternalInput")
    bias = nc.dram_tensor("bias", [MM, 2], dt.float32, kind="ExternalInput")
    # permuted row order (rl, sb, t): row r = 112*sb + 4*rl + t lives at
    # [rl, sb, t]; host un-permutes. Garbage rows 222/223 are (rl=27, sb=1,
    # t=2/3) and are dropped on the host. This makes each (img, dh) output a
    # single fully-contiguous-per-channel 796KB DMA with 7104B descriptors.
    # image-major so each (img, dh) DMA's 4 channels are DRAM-adjacent:
    # the DMA's DRAM AP merges to 2 dims (3-dim APs run HWDGE descriptor
    # generation ~3x slower: 102 vs 276 GB/s measured).
    # pair-interleaved: [pair, ch, rl, sb, t, img, j] so one DMA per
    # (pair, dh) covers 4 channels x 2 images = 1.59MB, fully merging to a
    # 1-dim DRAM AP with 14.2KB descriptors.
    yout = nc.dram_tensor("yout", [n_imgs // 2, ND, NRL, 2, NT, 2, WO],
                          dt.float32, kind="ExternalOutput")

    with tile.TileContext(nc) as tc:
        with (
            tc.tile_pool(name="const", bufs=1) as constp,
            tc.tile_pool(name="inp", bufs=3) as inp,
            tc.tile_pool(name="outp", bufs=3) as outp,
            tc.tile_pool(name="psum", bufs=8, space="PSUM") as psp,
        ):
            # per-(sb,t) weight tiles: first matmul gates on one 344KB DMA
            # (a single lw tile made it wait for the whole 2.75MB constant);
            # all 8 loads still emitted upfront, split across both rings in
            # the order pair 0 consumes them
            lwt = [[constp.tile([128, 3, 2, MM], io_dt, name=f"lw{s}{tt}")
                    for tt in range(NT)] for s in range(2)]
            for i, (s, tt) in enumerate(
                    [(s, tt) for s in range(2) for tt in range(NT)]):
                leng = nc.sync if i % 2 == 0 else nc.scalar
                leng.dma_start(lwt[s][tt][:], lw[:, s, tt, :, :, :])
            bias_sb = constp.tile([MM, 2], dt.float32)
            nc.scalar.dma_start(bias_sb[:], bias[:])

            def load_group(g):
                g8 = g * GSZ
                tiles = []
                for ti, (r0, nr) in enumerate(TILE_ROWS):
                    t = inp.tile([nr, GSZ, W_IN], io_dt, name=f"t{ti}",
                                 tag=f"t{ti}")
                    if ti == 0:
                        # per-image 2-dim DMAs on the HWDGE rings
                        for im in range(GSZ):
                            ieng = nc.sync if im % 2 == 0 else nc.scalar
                            ieng.dma_start(t[:, im, :],
                                           xin[g8 + im, r0:r0 + nr, :])
                    else:
                        # batched 3-dim load on the idle SWDGE queue; its
                        # slower descriptor-gen hides in the prefetch lead
                        nc.gpsimd.dma_start(
                            t[:],
                            xin[g8:g8 + GSZ, r0:r0 + nr, :].transpose([1, 0, 2]))
                    tiles.append(t)
                return tiles

            next_tiles = load_group(0)
            for g in range(n_groups):
                g8 = g * GSZ
                tiles = next_tiles
                for pr in range(GSZ // 2):
                    if pr == 1 and g + 1 < n_groups:
                        next_tiles = load_group(g + 1)
                    # acc[dh]: [112, img, sb, t, j]; per partition per image
                    # the (sb, t, j) block maps to 2x 4-consecutive-DRAM-rows
                    # acc[dh]: [112, sb, t, img, j]; per-partition free run
                    # (sb, t, img, j) = 3552 elems contiguous in DRAM
                    acc = [
                        outp.tile([MM, 2, NT, 2, WO], dt.float32,
                                  name=f"acc{dh}", tag=f"acc{dh}")
                        for dh in range(2)
                    ]
                    for dh in range(2):
                        for sb in range(2):
                            src = tiles[sb]
                            for tt in range(NT):
                                ps = psp.tile([MM, 2, WO], dt.float32,
                                              name="ps")
                                for v in range(3):
                                    nc.tensor.matmul(
                                        ps[:],
                                        lwt[sb][tt][:, v, dh, :],
                                        src[:, 2 * pr:2 * pr + 2, v:v + WO],
                                        start=(v == 0),
                                        stop=(v == 2),
                                    )
                                nc.vector.tensor_scalar_add(
                                    acc[dh][:, sb, tt, :, :],
                                    ps[:],
                                    bias_sb[:, dh:dh + 1],
                                )
                        # acc[dh] complete: one 1.59MB DMA for the pair
                        # (last pair: split into channel-pair halves across
                        # both rings to halve the final drain)
                        pair = g * (GSZ // 2) + pr
                        if pair == n_imgs // 2 - 1:
                            for hh in range(2):
                                heng = nc.sync if (dh + hh) % 2 == 0 \
                                    else nc.scalar
                                heng.dma_start(
                                    yout[pair,
                                         4 * dh + 2 * hh:4 * dh + 2 * hh + 2,
                                         :, :, :, :, :],
                                    acc[dh][56 * hh:56 * hh + 56])
                        else:
                            eng = (nc.sync if (pair + dh) % 2 == 0
                                   else nc.scalar)
                            eng.dma_start(
                                yout[pair, 4 * dh:4 * dh + 4, :, :, :, :, :],
                                acc[dh][:])

    nc.compile()
    return nc


def _get_prog(mode: str, n_imgs: int = C):
    key = (mode, n_imgs)
    if key not in _PROG_CACHE:
        _PROG_CACHE[key] = _build(mode, n_imgs)
    return _PROG_CACHE[key]


def _host_weights(W: np.ndarray, Bv: np.ndarray, mode: str):
    """LW[lr, sb, t, v, dh, d'*28+rl] = W[0, 4dh+d', u, v] where
    lr = (SB_BASE[sb] + 4*rl + t + u) - TILE_ROWS[sb][0], clipped to <128.
    bias[d'*28+rl, dh] = Bv[4dh+d']."""
    W = np.asarray(W, np.float32)
    LW = np.zeros((128, 2, NT, 3, 2, MM), np.float32)
    for sb in range(2):
        tile_base = TILE_ROWS[sb][0]
        out_base = SB_BASE[sb]
        for tt in range(NT):
            for v in range(3):
                for dh in range(2):
                    for dd in range(4):
                        for rl in range(NRL):
                            for u in range(3):
                                lr = out_base + 4 * rl + tt + u - tile_base
                                if 0 <= lr < 128:
                                    LW[lr, sb, tt, v, dh, dd * NRL + rl] = \
                                        W[0, 4 * dh + dd, u, v]
    bias = np.stack(
        [np.repeat(np.asarray(Bv[4 * dh:4 * dh + 4], np.float32), NRL)
         for dh in range(2)], axis=1)
    if mode == "bf16":
        import ml_dtypes
        LW = LW.astype(ml_dtypes.bfloat16)
    return np.ascontiguousarray(LW), np.ascontiguousarray(bias)


def _cast_in(x: np.ndarray, mode: str):
    if mode == "bf16":
        import ml_dtypes
        return np.ascontiguousarray(x).astype(ml_dtypes.bfloat16)
    return np.ascontiguousarray(x, np.float32)


def kernel(x, W, Bv, mode: str | None = None, _trace: bool = False):
    from concourse.bass_utils import run_bass_kernel_spmd

    mode = mode or os.environ.get("DCONV_MODE", "f32r")
    x = np.asarray(x, np.float32)
    W = np.asarray(W, np.float32)
    Bv = np.asarray(Bv, np.float32)

    nc = _get_prog(mode)
    LW, bias = _host_weights(W, Bv, mode)
    in_maps = [
        {"xin": _cast_in(x[k], mode), "lw": LW, "bias": bias}
        for k in range(NCORES)
    ]
    res = run_bass_kernel_spmd(nc, in_maps, core_ids=list(range(NCORES)),
                               trace=_trace)
    # yout is [pair, ch, rl, sb, t, img, j]; reorder to (d, pair, img) =
    # channels, (sb, rl, t) = row-major rows, drop the 2 pad rows.
    y = np.stack(
        [np.ascontiguousarray(
            np.asarray(res.results[k]["yout"]).transpose(1, 0, 5, 3, 2, 4, 6)
            .reshape(ND * C, 224, WO)[:, :HO, :]
        ) for k in range(NCORES)],
        axis=0,
    )
    if _trace:
        return y, res
    return y

